# revision 66
# baseline (speedup 1.0000x reference)
"""Trainium2 Bass kernel for nn_MultiHeadAttention_88536455840315.

Math notes (vs the jax reference):
  - The second einsum (log_probs[..., None] * attn) @ v factors to
    log_probs[..., None] * (attn @ v) because log_probs does not depend on
    the key index.  So only two big attention matmuls are needed.
  - Softmax is computed without max subtraction: dots ~ N(0,1) here, so
    exp(dots*scale) never overflows fp32.
  - sumexp is fused into the attn@v matmul as a ones column appended to V.

Sharding (8 cores): core c handles batch c//4 and query rows
(c%4)*512 .. +512 of that batch.  Each core computes the full K/V for its
batch (replicated within the 4-core group, no collectives).  The per-core
query offset is realized by rolling the batch rows host-side so that each
core's queries are always rows 0:512 (softmax is permutation-invariant
over keys, so rolling K/V order is exact).

x is transposed host-side, so X^T loads straight from DRAM with no PE
transposes.  All weights are prefetched in large DMAs during the V
projection.  V (+ones) and the exp(dots) tiles are bf16 (the attn@v
matmul runs bf16 at the same PE rate; halves their SBUF footprint); Q/K
and the dots stay float32r.
"""

import sys

if "/opt/trn_rl_repo" not in sys.path:
    sys.path.insert(0, "/opt/trn_rl_repo")

import numpy as np

import concourse.bass as bass
import concourse.mybir as mybir
import concourse.tile as tile
from concourse import bacc
from concourse import bass_utils
from concourse.masks import make_identity

F32 = mybir.dt.float32
F32R = mybir.dt.float32r
BF16 = mybir.dt.bfloat16
AF = mybir.ActivationFunctionType
ALU = mybir.AluOpType
AX = mybir.AxisListType

B, N, E = 2, 2048, 768
H, DH = 12, 64
HD = H * DH            # 768
NQ = 512               # query rows per core
SCALE = DH ** -0.5
LOG2PI = float(np.log(2.0 * np.pi))
CONST = -0.5 * DH * LOG2PI   # -32*log(2*pi)

NE = E // 128          # 6 chunks of the embedding dim
NN = N // 128          # 16 chunks of the sequence
NQT = NQ // 128        # 4 query tiles


def _ap3(t, offset_elems, mid, inner):
    """3D AP view [128, mid, inner] of tile t at an element offset."""
    return bass.AP(tensor=t.tensor, offset=t.offset + offset_elems,
                   ap=[t.ap[0], list(mid), list(inner)])


def _emit(tc):
    nc = tc.nc
    xT = nc.dram_tensor("xT", [E, N], BF16, kind="ExternalInput").ap()
    wqkv = nc.dram_tensor("wqkv", [E, 3 * HD], BF16, kind="ExternalInput").ap()
    wout = nc.dram_tensor("wout", [HD, E], BF16, kind="ExternalInput").ap()
    bout_t = nc.dram_tensor("bout", [E], F32, kind="ExternalInput")
    y = nc.dram_tensor("y", [NQ, E], F32, kind="ExternalOutput").ap()

    with tc.tile_pool(name="consts", bufs=1) as consts, \
         tc.tile_pool(name="big", bufs=1) as big:
        ident = consts.tile([128, 128], F32, name="ident", tag="ident")
        make_identity(nc, ident)
        ident_b = consts.tile([128, 128], BF16, name="identb", tag="identb")
        nc.vector.tensor_copy(ident_b, ident)
        scratch = consts.tile([1, 1], F32, name="scr", tag="scr")
        # preload the Exp and Ln activation tables while ACT is idle
        nc.scalar.activation(out=scratch, in_=ident[0:1, 0:1], func=AF.Exp)
        nc.scalar.activation(out=scratch, in_=ident[0:1, 0:1], func=AF.Ln)

        # persistent tensors: XT split [e][r] into [128, 512] pieces so the
        # serial DMA device can feed the V projection incrementally
        XTn = [[big.tile([128, 512], BF16, name=f"xt{i}_{r}", tag=f"xt{i}_{r}")
                for r in range(4)] for i in range(NE)]
        KT = [big.tile([128, N], F32R, name=f"kt{i}", tag=f"kt{i}")
              for i in range(NE)]
        VA = [big.tile([128, H, DH + 1], BF16, name=f"va{j}", tag=f"va{j}")
              for j in range(NN)]
        QT = [big.tile([128, NQ], F32R, name=f"qt{i}", tag=f"qt{i}")
              for i in range(NE)]
        PROD = big.tile([128, NQT, H, DH + 1], F32, name="prod", tag="prod")
        ACCS = big.tile([128, NQT, DH], F32, name="accs", tag="accs")
        ACCQ = big.tile([128, NQT, DH], F32, name="accq", tag="accq")
        bias = big.tile([128, E], F32, name="bias", tag="bias")
        WO = [big.tile([128, E], BF16, name=f"wo{c}", tag=f"wo{c}")
              for c in range(NE)]

        # ---- prefetch in consumption order ----
        # The sim's DMA transfer device is serial, so order = priority.
        # gpsimd (SWDGE) carries the XT stream; sync carries the weights.
        wv0_dma = []
        for r in range(4):
            for e in range(NE):
                nc.gpsimd.dma_start(
                    out=XTn[e][r],
                    in_=xT[e * 128:(e + 1) * 128, r * 512:(r + 1) * 512])
        with tc.tile_pool(name="wkp", bufs=1) as wkp:
            WK = [wkp.tile([128, HD], BF16, name=f"wk{e}", tag=f"wk{e}")
                  for e in range(NE)]
            QW = [wkp.tile([128, HD], BF16, name=f"qw{e}", tag=f"qw{e}")
                  for e in range(NE)]
            WV = [wkp.tile([128, HD], BF16, name=f"wv{e}", tag=f"wv{e}")
                  for e in range(NE)]
            # ones columns of VA (strided memset, cheap)
            for va in VA:
                nc.gpsimd.memset(
                    bass.AP(tensor=va.tensor, offset=va.offset + DH,
                            ap=[va.ap[0], [DH + 1, H]]), 1.0)

            # PSUM is readable only by PE/DVE/ACT; rotate PSUM->SBUF copies
            # between DVE and ACT except while ACT is saturated by exps.
            ceng = [nc.vector, nc.scalar]
            cp = 0

            def copy(dst, src):
                nonlocal cp
                eng = ceng[cp % len(ceng)]
                if eng is nc.scalar:
                    eng.copy(dst, src)
                else:
                    eng.tensor_copy(dst, src)
                cp += 1

            if True:
              with tc.tile_pool(name="pps", bufs=2, space="PSUM") as pps, \
                   tc.tile_pool(name="wvqp", bufs=1) as wvqp:
                for e in range(NE):
                    nc.sync.dma_start(
                        out=WV[e],
                        in_=wqkv[e * 128:(e + 1) * 128, 2 * HD:3 * HD])
                for e in range(NE):
                    nc.sync.dma_start(
                        out=QW[e], in_=wqkv[e * 128:(e + 1) * 128, 0:HD])
                for e in range(NE):
                    nc.sync.dma_start(
                        out=WK[e], in_=wqkv[e * 128:(e + 1) * 128, HD:2 * HD])
                for c in range(NE):
                    nc.sync.dma_start(
                        out=WO[c], in_=wout[c * 128:(c + 1) * 128, :])
                nc.sync.dma_start(out=bias, in_=bass.AP(
                    tensor=bout_t, offset=0, ap=[[0, 128], [1, E]]))

                # ---------------- V projection (last 2 blocks deferred
                # into head 0, where only its final attnv needs them) ------
                for nb in range(NN - 2):
                    psA = pps.tile([128, 512], F32, name="pp", tag="pp")
                    psB = pps.tile([128, 256], F32, name="ppb", tag="pp")
                    xs = [XTn[e][nb // 4][:, (nb % 4) * 128:
                                          (nb % 4 + 1) * 128]
                          for e in range(NE)]
                    for e in range(NE):
                        nc.tensor.matmul(
                            psA, xs[e], WV[e][:, 0:512],
                            start=(e == 0), stop=(e == NE - 1))
                    for e in range(NE):
                        nc.tensor.matmul(
                            psB, xs[e], WV[e][:, 512:768],
                            start=(e == 0), stop=(e == NE - 1))
                    va = VA[nb]
                    copy(_ap3(va, 0, [DH + 1, 8], [1, DH]),
                         psA.rearrange("p (h d) -> p h d", h=8))
                    copy(_ap3(va, 8 * (DH + 1), [DH + 1, 4], [1, DH]),
                         psB.rearrange("p (h d) -> p h d", h=4))

                # ---------------- Q^T projection (chunk 0 only) --------
                # heads 0/1 need only QT[0]; the rest is emitted after head 0
                # so the exp pipeline starts ~6us earlier.
                for qc in range(1):
                    ps = pps.tile([128, 512], F32, name="pp", tag="pp")
                    for e in range(NE):
                        nc.tensor.matmul(
                            ps, QW[e][:, qc * 128:(qc + 1) * 128],
                            XTn[e][0],
                            start=(e == 0), stop=(e == NE - 1))
                    copy(QT[qc], ps)

              # ---------- interleaved K projection + attention ----------
              # lag-2 software pipeline over 3 rotating PSUM slots; the next
              # K-projection chunk's matmuls are spread into the odd head's
              # exp-wait gaps instead of bunching at the chunk boundary.
              ceng = [nc.vector]
              with tc.tile_pool(name="slotp", bufs=3, space="PSUM") as slotp, \
                   tc.tile_pool(name="kpsp", bufs=1, space="PSUM") as kpsp, \
                   tc.tile_pool(name="nps", bufs=1, space="PSUM") as nps, \
                   tc.tile_pool(name="expp", bufs=6) as expp, \
                   tc.tile_pool(name="nsb", bufs=2) as nsb:

                    def kblock(kc, nb):
                        ps = kpsp.tile([128, 512], F32, name="kp", tag="kp")
                        for e in range(NE):
                            nc.tensor.matmul(
                                ps, WK[e][:, kc * 128:(kc + 1) * 128],
                                XTn[e][nb],
                                start=(e == 0), stop=(e == NE - 1))
                        nc.vector.tensor_copy(
                            KT[kc][:, nb * 512:(nb + 1) * 512], ps)

                    def vblock(nb):
                        xs = [XTn[e][nb // 4][:, (nb % 4) * 128:
                                              (nb % 4 + 1) * 128]
                              for e in range(NE)]
                        psA = kpsp.tile([128, 512], F32, name="vpA",
                                        tag="kp")
                        for e in range(NE):
                            nc.tensor.matmul(psA, xs[e], WV[e][:, 0:512],
                                             start=(e == 0),
                                             stop=(e == NE - 1))
                        copy(_ap3(VA[nb], 0, [DH + 1, 8], [1, DH]),
                             psA.rearrange("p (h d) -> p h d", h=8))
                        psB = kpsp.tile([128, 256], F32, name="vpB",
                                        tag="kp")
                        for e in range(NE):
                            nc.tensor.matmul(psB, xs[e], WV[e][:, 512:768],
                                             start=(e == 0),
                                             stop=(e == NE - 1))
                        copy(_ap3(VA[nb], 8 * (DH + 1), [DH + 1, 4],
                                  [1, DH]),
                             psB.rearrange("p (h d) -> p h d", h=4))

                    for nb in range(4):
                        kblock(0, nb)

                    for kc in range(NE):
                        for h in (2 * kc, 2 * kc + 1):
                            spread = kc + 1 < NE
                            kt = KT[h // 2]
                            pofs = (h % 2) * DH
                            qth = QT[h // 2][pofs:pofs + DH, :]
                            num_ps = nps.tile([DH + 1, NQ], F32, name="num",
                                              tag="num")
                            exs = []

                            def attnv(jj):
                                for k in range(2):
                                    jb = jj * 2 + k
                                    nc.tensor.matmul(num_ps,
                                                     VA[jb][:, h, :],
                                                     exs[jj][:, k, :],
                                                     start=(jb == 0),
                                                     stop=(jb == NN - 1))

                            for jj in range(8):
                                dt_ = slotp.tile([128, 2, NQ], F32,
                                                 name="dots", tag="slot")
                                for k in range(2):
                                    jb = jj * 2 + k
                                    nc.tensor.matmul(
                                        dt_[:, k, :],
                                        kt[pofs:pofs + DH,
                                           jb * 128:(jb + 1) * 128],
                                        qth, start=True, stop=True)
                                ex = expp.tile([128, 2, NQ], BF16,
                                               name="expd", tag="expd")
                                nc.scalar.activation(out=ex, in_=dt_,
                                                     func=AF.Exp, scale=SCALE)
                                exs.append(ex)
                                if jj >= 2:
                                    attnv(jj - 2)
                                if spread and jj % 4 == 1:
                                    kblock(kc + 1,
                                           (h % 2) * 2 + jj // 4)
                                if h == 0 and jj in (0, 2):
                                    vblock(NN - 2 + jj // 2)
                            attnv(6)
                            attnv(7)
                            numsb = nsb.tile([DH + 1, NQ], F32,
                                             name="numsb", tag="numsb",
                                             bufs=3)
                            nc.vector.tensor_copy(numsb, num_ps)
                            for qt in range(NQT):
                                # last head: the dots slots are free, so its
                                # transposes pipeline across 3 banks instead
                                # of serializing through the num bank
                                if h == H - 1:
                                    tp = slotp.tile([128, DH + 1], F32,
                                                    name="nt", tag="slot")
                                elif h == H - 2:
                                    tp = kpsp.tile([128, DH + 1], F32,
                                                   name="nt", tag="kp")
                                else:
                                    tp = nps.tile([128, DH + 1], F32,
                                                  name="nt", tag="num")
                                nc.tensor.transpose(
                                    tp, numsb[:, qt * 128:(qt + 1) * 128],
                                    ident[0:DH + 1, 0:DH + 1])
                                copy(PROD[:, qt, h, :], tp)
                            # normalize head h; accumulate sum / sum-of-sq
                            stq = H * (DH + 1)
                            rsh = nsb.tile([128, NQT], F32, name="rsh",
                                           tag="rsh", bufs=3)
                            nc.vector.reciprocal(rsh, bass.AP(
                                tensor=PROD.tensor,
                                offset=PROD.offset + h * (DH + 1) + DH,
                                ap=[PROD.ap[0], [stq, NQT]]))
                            pvh = bass.AP(tensor=PROD.tensor,
                                          offset=PROD.offset + h * (DH + 1),
                                          ap=[PROD.ap[0], [stq, NQT],
                                              [1, DH]])
                            rsh_bc = bass.AP(tensor=rsh.tensor,
                                             offset=rsh.offset,
                                             ap=[rsh.ap[0], [1, NQT],
                                                 [0, DH]])
                            nc.vector.tensor_tensor(out=pvh, in0=pvh,
                                                    in1=rsh_bc, op=ALU.mult)
                            if h == 0:
                                nc.gpsimd.tensor_copy(ACCS, pvh)
                                nc.gpsimd.tensor_tensor(out=ACCQ, in0=pvh,
                                                        in1=pvh, op=ALU.mult)
                                for qc in range(1, NE):
                                    ps = kpsp.tile([128, 512], F32,
                                                   name="qp", tag="kp")
                                    for e in range(NE):
                                        nc.tensor.matmul(
                                            ps,
                                            QW[e][:, qc * 128:(qc + 1) * 128],
                                            XTn[e][0],
                                            start=(e == 0),
                                            stop=(e == NE - 1))
                                    nc.vector.tensor_copy(QT[qc], ps)
                            elif h < H - 1:
                                sqh = nsb.tile([128, NQT, DH], F32,
                                               name="sqh", tag="sqh", bufs=3)
                                nc.gpsimd.tensor_tensor(out=sqh, in0=pvh,
                                                        in1=pvh, op=ALU.mult)
                                nc.gpsimd.tensor_tensor(out=ACCS, in0=ACCS,
                                                        in1=pvh, op=ALU.add)
                                nc.gpsimd.tensor_tensor(out=ACCQ, in0=ACCQ,
                                                        in1=sqh, op=ALU.add)

        # ---------------- statistics / log-prob weighting ----------------
        # All 4 query tiles processed in single wide ops over the fused
        # PROD tile [128, NQT, H, DH+1] (partitions = q mod 128).
        with tc.tile_pool(name="ohp", bufs=1) as ohp, \
             tc.tile_pool(name="wkp2", bufs=1) as wkp:
            ceng = [nc.vector, nc.scalar]
            deng = [nc.vector, nc.gpsimd]
            Pt = PROD
            # per-qt statistics so qt0's chain starts immediately
            mean = wkp.tile([128, NQT, DH], F32, name="mean", tag="mean")
            m2s = wkp.tile([128, NQT, DH], F32, name="m2s", tag="m2s")
            var = wkp.tile([128, NQT, DH], F32, name="var", tag="var")
            stq = H * (DH + 1)
            p11 = bass.AP(tensor=Pt.tensor,
                          offset=Pt.offset + (H - 1) * (DH + 1),
                          ap=[Pt.ap[0], [stq, NQT], [1, DH]])
            sq11 = wkp.tile([128, NQT, DH], F32, name="sq11", tag="sq11")
            for qt in range(NQT):
                p11q = bass.AP(tensor=Pt.tensor,
                               offset=Pt.offset + qt * H * (DH + 1)
                               + (H - 1) * (DH + 1),
                               ap=[Pt.ap[0], [1, 1], [1, DH]])
                deng[qt % 2].tensor_tensor(out=sq11[:, qt, :], in0=p11q,
                                           in1=p11q, op=ALU.mult)
            for qt in range(NQT):
                # mean = (ACCS_0..10 + p_11)/12 ; var likewise folds head 11
                nc.vector.scalar_tensor_tensor(
                    out=mean[:, qt, :], in0=ACCS[:, qt, :],
                    scalar=1.0, in1=bass.AP(
                        tensor=Pt.tensor,
                        offset=Pt.offset + qt * stq + (H - 1) * (DH + 1),
                        ap=[Pt.ap[0], [1, 1], [1, DH]]),
                    op0=ALU.mult, op1=ALU.add)
                nc.vector.tensor_scalar_mul(mean[:, qt, :], mean[:, qt, :],
                                            1.0 / H)
                nc.vector.scalar_tensor_tensor(
                    out=m2s[:, qt, :], in0=mean[:, qt, :],
                    scalar=H / (H - 1.0), in1=mean[:, qt, :],
                    op0=ALU.mult, op1=ALU.mult)
                nc.vector.tensor_tensor(out=var[:, qt, :],
                                        in0=ACCQ[:, qt, :],
                                        in1=sq11[:, qt, :], op=ALU.add)
                nc.vector.scalar_tensor_tensor(
                    out=var[:, qt, :], in0=var[:, qt, :],
                    scalar=1.0 / (H - 1), in1=m2s[:, qt, :],
                    op0=ALU.mult, op1=ALU.subtract)
            # tail per query-tile so the output projection can start on
            # qt 0 while qt 1-3 statistics still run
            with tc.tile_pool(name="ohtp", bufs=1) as ohtp, \
                 tc.tile_pool(name="finp", bufs=2) as finp, \
                 tc.tile_pool(name="tp2", bufs=2, space="PSUM") as tp2p, \
                 tc.tile_pool(name="fps", bufs=3, space="PSUM") as fps:
                for qt in range(NQT):
                    ve = deng[qt % 2]
                    pvq = bass.AP(tensor=Pt.tensor,
                                  offset=Pt.offset + qt * H * (DH + 1),
                                  ap=[Pt.ap[0], [DH + 1, H], [1, DH]])
                    varq = var[:, qt, :]
                    rvar = wkp.tile([128, DH], F32, name="rvar", tag="rvar",
                                    bufs=2)
                    nc.vector.reciprocal(rvar, varq)
                    lv = wkp.tile([128, DH], F32, name="lv", tag="lv", bufs=2)
                    S = wkp.tile([128, 1], F32, name="S", tag="S", bufs=2)
                    nc.scalar.activation(out=lv, in_=varq, func=AF.Ln,
                                         accum_out=S)
                    cs = wkp.tile([128, 1], F32, name="cs", tag="cs", bufs=2)
                    nc.gpsimd.tensor_scalar(out=cs, in0=S, scalar1=-1.0,
                                            scalar2=CONST, op0=ALU.mult,
                                            op1=ALU.add)
                    diff = wkp.tile([128, H, DH], F32, name="diff",
                                    tag="diff", bufs=2)
                    mean_bc = bass.AP(tensor=mean.tensor,
                                      offset=mean.offset + qt * DH,
                                      ap=[mean.ap[0], [0, H], [1, DH]])
                    ve.tensor_tensor(out=diff, in0=pvq, in1=mean_bc,
                                     op=ALU.subtract)
                    ve.tensor_tensor(out=diff, in0=diff, in1=diff,
                                     op=ALU.mult)
                    rvar_bc = bass.AP(tensor=rvar.tensor, offset=rvar.offset,
                                      ap=[rvar.ap[0], [0, H], [1, DH]])
                    ve.tensor_tensor(out=diff, in0=diff, in1=rvar_bc,
                                     op=ALU.mult)
                    lp0 = wkp.tile([128, H], F32, name="lp0", tag="lp0",
                                   bufs=2)
                    nc.vector.tensor_reduce(lp0, diff, op=ALU.add, axis=AX.X)
                    lp = wkp.tile([128, H], F32, name="lp", tag="lp", bufs=2)
                    nc.gpsimd.tensor_scalar(out=lp, in0=lp0, scalar1=0.25,
                                            scalar2=cs, op0=ALU.mult,
                                            op1=ALU.add)
                    oh = ohp.tile([128, H * DH], BF16, name=f"oh{qt}",
                                  tag=f"oh{qt}")
                    ohv = oh.rearrange("p (h d) -> p h d", h=H)
                    lp_bc = bass.AP(tensor=lp.tensor, offset=lp.offset,
                                    ap=[lp.ap[0], [1, H], [0, DH]])
                    ve.tensor_tensor(out=ohv, in0=pvq, in1=lp_bc,
                                     op=ALU.mult)
                    oht = ohtp.tile([128, NE, 128], BF16, name=f"oht{qt}",
                                    tag=f"oht{qt}")
                    for c in range(NE):
                        tp = tp2p.tile([128, 128], BF16, name="t2", tag="t2")
                        nc.tensor.transpose(
                            tp, oh[:, c * 128:(c + 1) * 128], ident_b)
                        copy(oht[:, c, :], tp)
                    psA = fps.tile([128, 512], F32, name="fA", tag="f")
                    psB = fps.tile([128, 256], F32, name="fB", tag="f")
                    for c in range(NE):
                        nc.tensor.matmul(psA, oht[:, c, :], WO[c][:, 0:512],
                                         start=(c == 0), stop=(c == NE - 1))
                    for c in range(NE):
                        nc.tensor.matmul(psB, oht[:, c, :], WO[c][:, 512:768],
                                         start=(c == 0), stop=(c == NE - 1))
                    fin = finp.tile([128, E], F32, name="fin", tag="fin")
                    nc.vector.tensor_tensor(out=fin[:, 0:512], in0=psA,
                                            in1=bias[:, 0:512], op=ALU.add)
                    nc.sync.dma_start(out=y[qt * 128:(qt + 1) * 128, 0:512],
                                      in_=fin[:, 0:512])
                    nc.vector.tensor_tensor(out=fin[:, 512:768], in0=psB,
                                            in1=bias[:, 512:768], op=ALU.add)
                    nc.scalar.dma_start(out=y[qt * 128:(qt + 1) * 128,
                                              512:768],
                                        in_=fin[:, 512:768])


_NC_CACHE = {}


def _get_nc():
    if "nc" not in _NC_CACHE:
        nc = bacc.Bacc("TRN2", target_bir_lowering=False, debug=False,
                       num_devices=8)
        with tile.TileContext(nc) as tc:
            _emit(tc)
        nc.compile()
        _NC_CACHE["nc"] = nc
    return _NC_CACHE["nc"]


def kernel(x, w_qkv, w_out, b_out):
    import ml_dtypes

    x = np.ascontiguousarray(x, dtype=np.float32)
    w_qkv_bf = np.ascontiguousarray(
        np.asarray(w_qkv, dtype=np.float32).astype(ml_dtypes.bfloat16))
    w_out_bf = np.ascontiguousarray(
        np.asarray(w_out, dtype=np.float32).astype(ml_dtypes.bfloat16))
    b_out = np.ascontiguousarray(b_out, dtype=np.float32)
    assert x.shape == (B, N, E)

    nc = _get_nc()
    in_maps = []
    for c in range(8):
        beta, qoff = c // 4, (c % 4) * NQ
        xTc = np.ascontiguousarray(
            np.roll(x[beta], -qoff, axis=0).T.astype(ml_dtypes.bfloat16))
        in_maps.append({"xT": xTc, "wqkv": w_qkv_bf, "wout": w_out_bf,
                        "bout": b_out})
    res = bass_utils.run_bass_kernel_spmd(nc, in_maps, core_ids=list(range(8)))
    out = np.empty((B, N, E), dtype=np.float32)
    for c in range(8):
        beta, qoff = c // 4, (c % 4) * NQ
        out[beta, qoff:qoff + NQ, :] = res.results[c]["y"]
    return out
